# revision 21
# baseline (speedup 1.0000x reference)
"""BiDAF attention-flow kernel for 8 Trainium2 NeuronCores (Bass/Tile).

Data-parallel over batch: B=32 -> 4 batches per core on 8 cores.

Math (per batch b):
  sim[i,j] = s_proj[i] + t_proj[j] + sum_d S[i,d]*wm[d]*T[j,d]
  A        = softmax_j(sim)            (row-constant s_proj cancels)
  source_target = A @ T
  w[i]     = exp(max_j sim[i,j]) ; tgt_attn = w / sum(w)
  target_source = tgt_attn @ S         (one row, broadcast over rows)
  out      = [S | source_target | S*source_target | S*target_source]

Device strategy (v3):
  - ONE sim pass only, in j-major (sim^T) layout: E^T = exp(sim_core + t_proj[j])
    with t_proj as the per-partition activation bias.  s_proj cancels in the
    softmax over j; for the max path it factors out of the monotone exp:
        w[i] = exp(s_proj[i]) * max_j E^T[j, i]
    so the row max comes from E^T via a 2-op DVE max tree + 4 PE transposes +
    one 3D reduce_max -- no second 512x512x400 matmul pass.
  - Projections on PE in row form; [1,512] rows -> [128,4] columns via a DRAM
    bounce (DMA may regroup a flat DRAM range; SBUF cannot cross partitions).
  - The target-source tail (w@S, broadcast, S*ts) of batch b executes one
    iteration later so the PE/ACT critical path of batch b+1 never waits on
    the long w-reduction chain.
  - Engine roles per batch: PE matmuls+transposes; ACT exp/stf; DVE max tree,
    reductions, reciprocals, wm-scale; GpSimd the output elementwise muls
    (off the critical path); sync issues input/bounce DMAs, scalar the output.
  - All HBM traffic bf16, fully contiguous p-major (host does pure layout
    marshalling); the exact S piece of the output is concatenated on host.
"""

import sys

import numpy as np
import ml_dtypes

# concourse is importable via the axon sitecustomize path; fall back to /opt.
try:
    import concourse.bass as bass
except ImportError:  # pragma: no cover
    sys.path.insert(0, "/opt/trn_rl_repo")
    import concourse.bass as bass

import concourse.mybir as mybir
import concourse.tile as tile
from concourse.bass import ts
from concourse.bass_utils import run_bass_kernel_spmd

B, LS, LT, D = 32, 512, 512, 400
N_CORES = 8
BL = B // N_CORES  # batches per core
F32 = mybir.dt.float32
BF16 = mybir.dt.bfloat16
EXP = mybir.ActivationFunctionType.Exp
AX = mybir.AxisListType.X

KP = [128, 128, 128, 16]  # contraction chunk sizes for d = 400


def _split_multi_waits(nc: bass.Bass) -> None:
    """This walrus build encodes at most ONE sync-wait per instruction
    ("Too many sync wait commands" in setupSyncWait).  Tile's wait pass can
    attach several sem-waits to one instruction; hoist the extras onto
    same-engine NoOp carriers immediately before it (the NX sequencer
    executes the waits in order, so semantics are identical)."""
    ctr = 0
    for fn in nc.m.functions:
        for bb in fn.blocks:
            if not any(
                i.sync_info is not None and len(i.sync_info.on_wait) > 1
                for i in bb.instructions
            ):
                continue
            new_insts = []
            for inst in bb.instructions:
                si = inst.sync_info
                if si is not None and len(si.on_wait) > 1:
                    waits = list(si.on_wait)
                    for w in waits[:-1]:
                        ctr += 1
                        nop = mybir.InstNoOp(
                            name=f"splitw-{ctr}",
                            engine=inst.engine,
                            sync_info=mybir.SyncInfo(on_wait=[w], on_update=[]),
                            bass_nofuse=True,
                        )
                        nc.register_instruction(nop, overwrite=True)
                        new_insts.append(nop)
                    del si.on_wait[:-1]
                new_insts.append(inst)
            bb.instructions[:] = new_insts


def build_program() -> bass.Bass:
    nc = bass.Bass("TRN2", target_bir_lowering=False, debug=False)

    # Per-core DRAM I/O -- everything p-major so every DMA is contiguous.
    rows_h = nc.dram_tensor("rows", [BL, 128, 2, 4, 404], BF16, kind="ExternalInput").ap()
    tmaj_h = nc.dram_tensor("tmaj", [BL, 128, 2, 3, 512], BF16, kind="ExternalInput").ap()
    tails_h = nc.dram_tensor("tails", [BL, 2, 16, 512], BF16, kind="ExternalInput").ap()
    wcols_h = nc.dram_tensor("wcols", [128, 8], BF16, kind="ExternalInput").ap()
    wmf_h = nc.dram_tensor("wmf", [128, 4], F32, kind="ExternalInput").ap()
    eye_h = nc.dram_tensor("eye", [128, 128], BF16, kind="ExternalInput").ap()
    onesb_h = nc.dram_tensor("onesb", [1, 128], BF16, kind="ExternalInput").ap()
    out_h = nc.dram_tensor("outp", [BL, 128, 3, 4, 400], BF16, kind="ExternalOutput").ap()

    with tile.TileContext(nc) as tc:
        with (
            tc.tile_pool(name="singles", bufs=1) as singles,
            tc.tile_pool(name="pin", bufs=3) as pin,
            tc.tile_pool(name="pet", bufs=2) as pet,
            tc.tile_pool(name="pout", bufs=2) as pout,
            tc.tile_pool(name="ptiny", bufs=3) as ptiny,
            tc.tile_pool(name="pbig_ps", bufs=4, space="PSUM") as pbig_ps,
            tc.tile_pool(name="ptp_ps", bufs=1, space="PSUM") as ptp_ps,
            tc.tile_pool(name="psml_ps", bufs=1, space="PSUM") as psml_ps,
            tc.tile_pool(name="pdram", bufs=2, space="DRAM") as pdram,
        ):
            wcols = singles.tile([128, 8], BF16)
            wmf = singles.tile([128, 4], F32)
            eye = singles.tile([128, 128], BF16)
            onesb = singles.tile([1, 128], BF16)

            def dma_consts():
                nc.scalar.dma_start(out=wcols[:], in_=wcols_h)
                nc.scalar.dma_start(out=wmf[:], in_=wmf_h)
                nc.scalar.dma_start(out=eye[:], in_=eye_h)
                nc.scalar.dma_start(out=onesb[:], in_=onesb_h)

            st = {}  # per-batch state dict

            def dma_in(b):
                tmaj = pin.tile([128, 2, 4, 512], BF16, tag="tmaj")
                nc.sync.dma_start(out=tmaj[:, 0, 0:3, :], in_=tmaj_h[b, :, 0])
                nc.sync.dma_start(out=tmaj[0:16, 0, 3, :], in_=tails_h[b, 0])
                nc.sync.dma_start(out=tmaj[:, 1, 0:3, :], in_=tmaj_h[b, :, 1])
                nc.sync.dma_start(out=tmaj[0:16, 1, 3, :], in_=tails_h[b, 1])
                rows = pin.tile([128, 2, 4, 404], BF16, tag="rows")
                nc.sync.dma_start(out=rows[:], in_=rows_h[b])
                st[b] = {
                    "srow": rows[:, 0],
                    "trow": rows[:, 1],
                    "stt": tmaj[:, 0],
                    "ttt": tmaj[:, 1],
                }

            def get_outt(b):
                if "outt" not in st[b]:
                    st[b]["outt"] = pout.tile(
                        [128, 3, 4, 400], BF16, tag="outt", name=f"outt{b}"
                    )
                return st[b]["outt"]

            def proj_pe(b):
                """s_proj/t_proj rows on PE (before ttt gets wm-scaled)."""
                s = st[b]
                pp = psml_ps.tile([64, 512], F32, tag="pp")
                for kc in range(4):
                    p = KP[kc]
                    nc.tensor.matmul(
                        pp[0:1, :],
                        lhsT=wcols[0:p, kc : kc + 1],
                        rhs=s["stt"][0:p, kc, :],
                        start=(kc == 0),
                        stop=(kc == 3),
                    )
                for kc in range(4):
                    p = KP[kc]
                    nc.tensor.matmul(
                        pp[32:33, :],
                        lhsT=wcols[0:p, 4 + kc : 5 + kc],
                        rhs=s["ttt"][0:p, kc, :],
                        start=(kc == 0),
                        stop=(kc == 3),
                    )
                s["pp"] = pp

            def proj_fix(b):
                """Rows -> per-partition columns via a DRAM bounce."""
                s = st[b]
                prow = ptiny.tile([1, 1024], F32, tag="prow")
                nc.scalar.copy(prow[0:1, 0:512], s["pp"][0:1, :])
                nc.scalar.copy(prow[0:1, 512:1024], s["pp"][32:33, :])
                pscr = pdram.tile([1, 1024], F32, tag="pscr")
                nc.scalar.dma_start(out=pscr[:], in_=prow[:])
                pcols = ptiny.tile([128, 8], F32, tag="pcols")
                nc.scalar.dma_start(
                    out=pcols[:],
                    in_=pscr[0].rearrange("(r jc p) -> p (r jc)", p=128, jc=4),
                )
                espt = ptiny.tile([128, 4], F32, tag="espt")
                nc.scalar.activation(espt[:], pcols[:, 0:4], EXP)
                s["pcols"] = pcols
                s["espt"] = espt

            def wm_scale(b):
                """ttts = ttt * wm (separate tile: runs concurrent w/ proj_pe)."""
                ttt = st[b]["ttt"]
                ttts = pin.tile([128, 4, 512], BF16, tag="ttts")
                for kc in range(4):
                    p = KP[kc]
                    nc.vector.tensor_scalar_mul(
                        ttts[0:p, kc, :], ttt[0:p, kc, :], wmf[0:p, kc : kc + 1]
                    )
                st[b]["ttts"] = ttts

            def sim_pe(b):
                s = st[b]
                et = pet.tile([128, 4, 512], BF16, tag="et")
                s["et"] = et
                for jc in range(4):
                    ps = pbig_ps.tile([128, 512], F32, tag="big", name=f"ps{jc}")
                    for kc in range(4):
                        p = KP[kc]
                        nc.tensor.matmul(
                            ps[:],
                            lhsT=s["ttts"][0:p, kc, ts(jc, 128)],
                            rhs=s["stt"][0:p, kc, :],
                            start=(kc == 0),
                            stop=(kc == 3),
                        )
                    nc.scalar.activation(
                        et[:, jc, :], ps[:], EXP, bias=s["pcols"][:, 4 + jc : 5 + jc]
                    )

            def max_tree(b):
                s = st[b]
                et = s["et"]
                m2 = pet.tile([128, 2, 512], BF16, tag="m2")
                nc.vector.tensor_max(m2[:], et[:, 0:2, :], et[:, 2:4, :])
                m = pet.tile([128, 512], BF16, tag="m")
                nc.vector.tensor_max(m[:], m2[:, 0, :], m2[:, 1, :])
                s["m"] = m

            def row_max(b):
                """PE transposes + one 3D reduce: w = max_j E^T * e^s_proj."""
                s = st[b]
                tps = ptp_ps.tile([128, 4, 128], BF16, tag="tp")
                for ic in range(4):
                    nc.tensor.transpose(
                        tps[:, ic, :], s["m"][:, ts(ic, 128)], eye[:]
                    )
                mx = ptiny.tile([128, 4], F32, tag="mx")
                nc.vector.reduce_max(mx[:], tps[:], axis=AX)
                wtile = ptiny.tile([128, 4], BF16, tag="wtile")
                nc.vector.tensor_mul(wtile[:], mx[:], s["espt"][:])
                s["wtile"] = wtile

            def at_pass(b):
                """A@[T|1] + normalization (stf on ACT) + S*stf (gpsimd)."""
                s = st[b]
                et, trow, srow = s["et"], s["trow"], s["srow"]
                outt = get_outt(b)
                rinv = ptiny.tile([128, 4], F32, tag="rinv")
                for ic in range(4):
                    po = pbig_ps.tile([128, 512], F32, tag="big", name=f"po{ic}")
                    for jc in range(4):
                        nc.tensor.matmul(
                            po[:, 0:401],
                            lhsT=et[:, jc, ts(ic, 128)],
                            rhs=trow[:, jc, 0:401],
                            start=(jc == 0),
                            stop=(jc == 3),
                        )
                    nc.vector.reciprocal(rinv[:, ic : ic + 1], po[:, 400:401])
                    nc.scalar.mul(
                        outt[:, 0, ic, :], po[:, 0:400], rinv[:, ic : ic + 1]
                    )
                nc.scalar.dma_start(out=out_h[b, :, 0:1], in_=outt[:, 0:1])
                # S * source_target in one 3D op (DVE)
                nc.vector.tensor_mul(
                    outt[:, 1, :, :], srow[:, :, 0:400], outt[:, 0, :, :]
                )
                nc.scalar.dma_start(out=out_h[b, :, 1:2], in_=outt[:, 1:2])

            def ts_tail_a(b):
                """w @ [S|1] (PE) -- emitted right after the next sim pass."""
                s = st[b]
                ps_ts = psml_ps.tile([1, 512], F32, tag="sml")
                for ic in range(4):
                    nc.tensor.matmul(
                        ps_ts[0:1, 0:401],
                        lhsT=s["wtile"][:, ic : ic + 1],
                        rhs=s["srow"][:, ic, 0:401],
                        start=(ic == 0),
                        stop=(ic == 3),
                    )
                s["ps_ts"] = ps_ts

            def ts_tail_b(b):
                """normalize + ones-broadcast + S*ts (after proj mm's, so the
                PE does not stall on the DVE rts/tsn chain)."""
                s = st[b]
                ps_ts = s["ps_ts"]
                rts = ptiny.tile([1, 1], F32, tag="rts")
                nc.vector.reciprocal(rts[:], ps_ts[0:1, 400:401])
                tsn = ptiny.tile([1, 400], BF16, tag="tsn")
                nc.vector.tensor_scalar_mul(tsn[:], ps_ts[0:1, 0:400], rts[:])
                ps_tsb = ptp_ps.tile([128, 400], F32, tag="tsb")
                nc.tensor.matmul(
                    ps_tsb[:], lhsT=onesb[:], rhs=tsn[:], start=True, stop=True
                )
                tsb = pout.tile([128, 400], BF16, tag="tsb")
                nc.vector.tensor_copy(tsb[:], ps_tsb[:])
                nc.vector.tensor_mul(
                    get_outt(b)[:, 2, :, :],
                    s["srow"][:, :, 0:400],
                    tsb[:].rearrange("p (o c) -> p o c", o=1).broadcast_to(
                        [128, 4, 400]
                    ),
                )

            def dma_out(b):
                nc.scalar.dma_start(out=out_h[b, :, 2:3], in_=st[b]["outt"][:, 2:3])
                del st[b]

            # ---- schedule ----
            dma_in(0)
            dma_consts()
            proj_pe(0)
            proj_fix(0)
            wm_scale(0)
            for b in range(BL):
                if b + 1 < BL:
                    dma_in(b + 1)
                sim_pe(b)  # PE + ACT exp
                if b > 0:
                    ts_tail_a(b - 1)  # w@S of previous batch (PE, deps ready)
                if b + 1 < BL:
                    proj_pe(b + 1)
                    proj_fix(b + 1)
                if b > 0:
                    ts_tail_b(b - 1)  # broadcast+muls (DVE chain now hidden)
                max_tree(b)  # DVE
                if b + 1 < BL:
                    wm_scale(b + 1)  # DVE (own tile; after tree(b) to avoid HoL block)
                row_max(b)  # PE transposes + DVE reduce
                if b == BL - 1:
                    ts_tail_a(b)
                    ts_tail_b(b)  # no next iteration: run now
                at_pass(b)  # PE A@T + ACT stf + DVE muls
                if b > 0:
                    dma_out(b - 1)
            dma_out(BL - 1)
    return nc


_NC_CACHE: list = []


def _get_program() -> bass.Bass:
    if not _NC_CACHE:
        nc = build_program()
        _split_multi_waits(nc)
        _NC_CACHE.append(nc)
    return _NC_CACHE[0]


def _host_shards(S: np.ndarray, T: np.ndarray, w: np.ndarray):
    """Build per-core input maps (pure layout marshalling, no math)."""
    BF = ml_dtypes.bfloat16
    ws, wt, wm = w[:D], w[D : 2 * D], w[2 * D :]
    wcols = np.zeros((128, 8), np.float32)
    wmf = np.zeros((128, 4), np.float32)
    for kc in range(4):
        p = KP[kc]
        wcols[0:p, kc] = ws[kc * 128 : kc * 128 + p]
        wcols[0:p, 4 + kc] = wt[kc * 128 : kc * 128 + p]
        wmf[0:p, kc] = wm[kc * 128 : kc * 128 + p]
    wcols = wcols.astype(BF)
    eye = np.eye(128, dtype=BF)
    onesb = np.ones((1, 128), BF)

    def rows_pmajor(X):  # [bl, 512, 400] -> [bl, 128, 4, 404], col 400 = 1.0
        bl = X.shape[0]
        out = np.zeros((bl, 4, 128, 404), np.float32)
        out[:, :, :, 0:400] = X.reshape(bl, 4, 128, 400)
        out[:, :, :, 400] = 1.0
        return out.transpose(0, 2, 1, 3)

    def t_pmajor(X):  # [bl, 512, 400] -> big [bl, 128, 3, 512] + tail [bl,16,512]
        bl = X.shape[0]
        xt = X.transpose(0, 2, 1)  # [bl, 400, 512]
        big = xt[:, 0:384, :].reshape(bl, 3, 128, 512).transpose(0, 2, 1, 3)
        tail = xt[:, 384:400, :]
        return big, tail

    in_maps = []
    for c in range(N_CORES):
        Sb = S[c * BL : (c + 1) * BL]
        Tb = T[c * BL : (c + 1) * BL]
        rows = np.stack([rows_pmajor(Sb), rows_pmajor(Tb)], axis=2)  # [bl,128,2,4,404]
        stbig, st3 = t_pmajor(Sb)
        ttbig, tt3 = t_pmajor(Tb)
        tmaj = np.stack([stbig, ttbig], axis=2)  # [bl, 128, 2, 3, 512]
        tails = np.stack([st3, tt3], axis=1)  # [bl, 2, 16, 512]
        in_maps.append(
            {
                "rows": np.ascontiguousarray(rows).astype(BF),
                "tmaj": np.ascontiguousarray(tmaj).astype(BF),
                "tails": np.ascontiguousarray(tails).astype(BF),
                "wcols": wcols,
                "wmf": wmf,
                "eye": eye,
                "onesb": onesb,
            }
        )
    return in_maps


def kernel(source_embedding, target_embedding, w_sim, **run_kwargs):
    S = np.asarray(source_embedding, dtype=np.float32)
    T = np.asarray(target_embedding, dtype=np.float32)
    w = np.asarray(w_sim, dtype=np.float32)
    assert S.shape == (B, LS, D) and T.shape == (B, LT, D) and w.shape == (3 * D,)

    nc = _get_program()
    in_maps = _host_shards(S, T, w)
    res = run_bass_kernel_spmd(nc, in_maps, core_ids=list(range(N_CORES)), **run_kwargs)

    out = np.empty((B, LS, 4 * D), np.float32)
    out[:, :, 0:D] = S  # exact copy of the input piece (host concat)
    for c in range(N_CORES):
        piece = np.asarray(res.results[c]["outp"])  # [BL, 128, 3, 4, 400]
        sl = slice(c * BL, (c + 1) * BL)
        for q in range(3):
            out[sl, :, (q + 1) * D : (q + 2) * D] = (
                piece[:, :, q]
                .transpose(0, 2, 1, 3)
                .reshape(BL, LS, D)
                .astype(np.float32)
            )
    if run_kwargs:
        kernel.last_results = res  # expose profile info to test harness
    return out


# revision 23
# speedup vs baseline: 1.0577x; 1.0577x over previous
"""BiDAF attention-flow kernel for 8 Trainium2 NeuronCores (Bass/Tile).

Data-parallel over batch: B=32 -> 4 batches per core on 8 cores.

Math (per batch b):
  sim[i,j] = s_proj[i] + t_proj[j] + sum_d S[i,d]*wm[d]*T[j,d]
  A        = softmax_j(sim)            (row-constant s_proj cancels)
  source_target = A @ T
  w[i]     = exp(max_j sim[i,j]) ; tgt_attn = w / sum(w)
  target_source = tgt_attn @ S         (one row, broadcast over rows)
  out      = [S | source_target | S*source_target | S*target_source]

Device strategy (v3):
  - ONE sim pass only, in j-major (sim^T) layout: E^T = exp(sim_core + t_proj[j])
    with t_proj as the per-partition activation bias.  s_proj cancels in the
    softmax over j; for the max path it factors out of the monotone exp:
        w[i] = exp(s_proj[i]) * max_j E^T[j, i]
    so the row max comes from E^T via a 2-op DVE max tree + 4 PE transposes +
    one 3D reduce_max -- no second 512x512x400 matmul pass.
  - Projections on PE in row form; [1,512] rows -> [128,4] columns via a DRAM
    bounce (DMA may regroup a flat DRAM range; SBUF cannot cross partitions).
  - The target-source tail (w@S, broadcast, S*ts) of batch b executes one
    iteration later so the PE/ACT critical path of batch b+1 never waits on
    the long w-reduction chain.
  - Engine roles per batch: PE matmuls+transposes; ACT exp/stf; DVE max tree,
    reductions, reciprocals, wm-scale; GpSimd the output elementwise muls
    (off the critical path); sync issues input/bounce DMAs, scalar the output.
  - All HBM traffic bf16, fully contiguous p-major (host does pure layout
    marshalling); the exact S piece of the output is concatenated on host.
"""

import sys

import numpy as np
import ml_dtypes

# concourse is importable via the axon sitecustomize path; fall back to /opt.
try:
    import concourse.bass as bass
except ImportError:  # pragma: no cover
    sys.path.insert(0, "/opt/trn_rl_repo")
    import concourse.bass as bass

import concourse.mybir as mybir
import concourse.tile as tile
from concourse.bass import ts
from concourse.bass_utils import run_bass_kernel_spmd

B, LS, LT, D = 32, 512, 512, 400
N_CORES = 8
BL = B // N_CORES  # batches per core
F32 = mybir.dt.float32
BF16 = mybir.dt.bfloat16
EXP = mybir.ActivationFunctionType.Exp
AX = mybir.AxisListType.X

KP = [128, 128, 128, 16]  # contraction chunk sizes for d = 400


def _split_multi_waits(nc: bass.Bass) -> None:
    """This walrus build encodes at most ONE sync-wait per instruction
    ("Too many sync wait commands" in setupSyncWait).  Tile's wait pass can
    attach several sem-waits to one instruction; hoist the extras onto
    same-engine NoOp carriers immediately before it (the NX sequencer
    executes the waits in order, so semantics are identical)."""
    ctr = 0
    for fn in nc.m.functions:
        for bb in fn.blocks:
            if not any(
                i.sync_info is not None and len(i.sync_info.on_wait) > 1
                for i in bb.instructions
            ):
                continue
            new_insts = []
            for inst in bb.instructions:
                si = inst.sync_info
                if si is not None and len(si.on_wait) > 1:
                    waits = list(si.on_wait)
                    for w in waits[:-1]:
                        ctr += 1
                        nop = mybir.InstNoOp(
                            name=f"splitw-{ctr}",
                            engine=inst.engine,
                            sync_info=mybir.SyncInfo(on_wait=[w], on_update=[]),
                            bass_nofuse=True,
                        )
                        nc.register_instruction(nop, overwrite=True)
                        new_insts.append(nop)
                    del si.on_wait[:-1]
                new_insts.append(inst)
            bb.instructions[:] = new_insts


def build_program() -> bass.Bass:
    nc = bass.Bass("TRN2", target_bir_lowering=False, debug=False)

    # Per-core DRAM I/O -- everything p-major so every DMA is contiguous.
    rows_h = nc.dram_tensor("rows", [BL, 128, 2, 4, 404], BF16, kind="ExternalInput").ap()
    tmaj_h = nc.dram_tensor("tmaj", [BL, 128, 2, 3, 512], BF16, kind="ExternalInput").ap()
    tails_h = nc.dram_tensor("tails", [BL, 2, 16, 512], BF16, kind="ExternalInput").ap()
    wcols_h = nc.dram_tensor("wcols", [128, 8], BF16, kind="ExternalInput").ap()
    wmf_h = nc.dram_tensor("wmf", [128, 4], F32, kind="ExternalInput").ap()
    eye_h = nc.dram_tensor("eye", [128, 128], BF16, kind="ExternalInput").ap()
    onesb_h = nc.dram_tensor("onesb", [1, 128], BF16, kind="ExternalInput").ap()
    out_h = nc.dram_tensor("outp", [BL, 128, 3, 4, 400], BF16, kind="ExternalOutput").ap()

    with tile.TileContext(nc) as tc:
        with (
            tc.tile_pool(name="singles", bufs=1) as singles,
            tc.tile_pool(name="pin", bufs=4) as pin,
            tc.tile_pool(name="pet", bufs=3) as pet,
            tc.tile_pool(name="pout", bufs=3) as pout,
            tc.tile_pool(name="ptiny", bufs=4) as ptiny,
            tc.tile_pool(name="pbig_ps", bufs=4, space="PSUM") as pbig_ps,
            tc.tile_pool(name="ptp_ps", bufs=1, space="PSUM") as ptp_ps,
            tc.tile_pool(name="psml_ps", bufs=1, space="PSUM") as psml_ps,
            tc.tile_pool(name="pdram", bufs=2, space="DRAM") as pdram,
        ):
            wcols = singles.tile([128, 8], BF16)
            wmf = singles.tile([128, 4], F32)
            eye = singles.tile([128, 128], BF16)
            onesb = singles.tile([1, 128], BF16)

            def dma_consts():
                nc.scalar.dma_start(out=wcols[:], in_=wcols_h)
                nc.scalar.dma_start(out=wmf[:], in_=wmf_h)
                nc.scalar.dma_start(out=eye[:], in_=eye_h)
                nc.scalar.dma_start(out=onesb[:], in_=onesb_h)

            st = {}  # per-batch state dict

            def dma_in(b):
                tmaj = pin.tile([128, 2, 4, 512], BF16, tag="tmaj")
                nc.sync.dma_start(out=tmaj[:, :, 0:3, :], in_=tmaj_h[b])
                nc.sync.dma_start(
                    out=tmaj[0:16, :, 3, :],
                    in_=tails_h[b].rearrange("t p f -> p t f"),
                )
                rows = pin.tile([128, 2, 4, 404], BF16, tag="rows")
                nc.sync.dma_start(out=rows[:], in_=rows_h[b])
                st[b] = {
                    "srow": rows[:, 0],
                    "trow": rows[:, 1],
                    "stt": tmaj[:, 0],
                    "ttt": tmaj[:, 1],
                }

            def get_outt(b):
                if "outt" not in st[b]:
                    st[b]["outt"] = pout.tile(
                        [128, 3, 4, 400], BF16, tag="outt", name=f"outt{b}"
                    )
                return st[b]["outt"]

            def proj_pe(b):
                """s_proj/t_proj rows on PE (before ttt gets wm-scaled)."""
                s = st[b]
                pp = psml_ps.tile([64, 512], F32, tag="pp")
                for kc in range(4):
                    p = KP[kc]
                    nc.tensor.matmul(
                        pp[0:1, :],
                        lhsT=wcols[0:p, kc : kc + 1],
                        rhs=s["stt"][0:p, kc, :],
                        start=(kc == 0),
                        stop=(kc == 3),
                    )
                for kc in range(4):
                    p = KP[kc]
                    nc.tensor.matmul(
                        pp[32:33, :],
                        lhsT=wcols[0:p, 4 + kc : 5 + kc],
                        rhs=s["ttt"][0:p, kc, :],
                        start=(kc == 0),
                        stop=(kc == 3),
                    )
                s["pp"] = pp

            def proj_fix(b):
                """Rows -> per-partition columns via a DRAM bounce."""
                s = st[b]
                prow = ptiny.tile([1, 1024], F32, tag="prow")
                nc.scalar.copy(prow[0:1, 0:512], s["pp"][0:1, :])
                nc.scalar.copy(prow[0:1, 512:1024], s["pp"][32:33, :])
                pscr = pdram.tile([1, 1024], F32, tag="pscr")
                nc.sync.dma_start(out=pscr[:], in_=prow[:])
                pcols = ptiny.tile([128, 8], F32, tag="pcols")
                nc.sync.dma_start(
                    out=pcols[:],
                    in_=pscr[0].rearrange("(r jc p) -> p (r jc)", p=128, jc=4),
                )
                espt = ptiny.tile([128, 4], F32, tag="espt")
                nc.scalar.activation(espt[:], pcols[:, 0:4], EXP)
                s["pcols"] = pcols
                s["espt"] = espt

            def wm_scale(b):
                """ttts = ttt * wm (separate tile: runs concurrent w/ proj_pe)."""
                ttt = st[b]["ttt"]
                ttts = pin.tile([128, 4, 512], BF16, tag="ttts")
                for kc in range(4):
                    p = KP[kc]
                    nc.vector.tensor_scalar_mul(
                        ttts[0:p, kc, :], ttt[0:p, kc, :], wmf[0:p, kc : kc + 1]
                    )
                st[b]["ttts"] = ttts

            def sim_pe(b):
                s = st[b]
                et = pet.tile([128, 4, 512], BF16, tag="et")
                s["et"] = et
                for jc in range(4):
                    ps = pbig_ps.tile([128, 512], F32, tag="big", name=f"ps{jc}")
                    for kc in range(4):
                        p = KP[kc]
                        nc.tensor.matmul(
                            ps[:],
                            lhsT=s["ttts"][0:p, kc, ts(jc, 128)],
                            rhs=s["stt"][0:p, kc, :],
                            start=(kc == 0),
                            stop=(kc == 3),
                        )
                    nc.scalar.activation(
                        et[:, jc, :], ps[:], EXP, bias=s["pcols"][:, 4 + jc : 5 + jc]
                    )

            def max_tree(b):
                s = st[b]
                et = s["et"]
                m2 = pet.tile([128, 2, 512], BF16, tag="m2")
                nc.vector.tensor_max(m2[:], et[:, 0:2, :], et[:, 2:4, :])
                m = pet.tile([128, 512], BF16, tag="m")
                nc.vector.tensor_max(m[:], m2[:, 0, :], m2[:, 1, :])
                s["m"] = m

            def row_max(b):
                """PE transposes + one 3D reduce: w = max_j E^T * e^s_proj."""
                s = st[b]
                tps = ptp_ps.tile([128, 4, 128], BF16, tag="tp")
                for ic in range(4):
                    nc.tensor.transpose(
                        tps[:, ic, :], s["m"][:, ts(ic, 128)], eye[:]
                    )
                mx = ptiny.tile([128, 4], F32, tag="mx")
                nc.vector.reduce_max(mx[:], tps[:], axis=AX)
                wtile = ptiny.tile([128, 4], BF16, tag="wtile")
                nc.vector.tensor_mul(wtile[:], mx[:], s["espt"][:])
                s["wtile"] = wtile

            def at_pass(b):
                """A@[T|1] + normalization (stf on ACT) + S*stf (gpsimd)."""
                s = st[b]
                et, trow, srow = s["et"], s["trow"], s["srow"]
                outt = get_outt(b)
                rinv = ptiny.tile([128, 4], F32, tag="rinv")
                for ic in range(4):
                    po = pbig_ps.tile([128, 512], F32, tag="big", name=f"po{ic}")
                    for jc in range(4):
                        nc.tensor.matmul(
                            po[:, 0:401],
                            lhsT=et[:, jc, ts(ic, 128)],
                            rhs=trow[:, jc, 0:401],
                            start=(jc == 0),
                            stop=(jc == 3),
                        )
                    nc.vector.reciprocal(rinv[:, ic : ic + 1], po[:, 400:401])
                    nc.scalar.mul(
                        outt[:, 0, ic, :], po[:, 0:400], rinv[:, ic : ic + 1]
                    )
                nc.scalar.dma_start(out=out_h[b, :, 0:1], in_=outt[:, 0:1])
                # S * source_target in one 3D op (DVE)
                nc.vector.tensor_mul(
                    outt[:, 1, :, :], srow[:, :, 0:400], outt[:, 0, :, :]
                )
                nc.scalar.dma_start(out=out_h[b, :, 1:2], in_=outt[:, 1:2])

            def ts_tail_a(b):
                """w @ [S|1] (PE) -- emitted right after the next sim pass."""
                s = st[b]
                ps_ts = psml_ps.tile([1, 512], F32, tag="sml")
                for ic in range(4):
                    nc.tensor.matmul(
                        ps_ts[0:1, 0:401],
                        lhsT=s["wtile"][:, ic : ic + 1],
                        rhs=s["srow"][:, ic, 0:401],
                        start=(ic == 0),
                        stop=(ic == 3),
                    )
                s["ps_ts"] = ps_ts

            def ts_tail_b(b):
                """normalize + ones-broadcast + S*ts (after proj mm's, so the
                PE does not stall on the DVE rts/tsn chain)."""
                s = st[b]
                ps_ts = s["ps_ts"]
                rts = ptiny.tile([1, 1], F32, tag="rts")
                nc.vector.reciprocal(rts[:], ps_ts[0:1, 400:401])
                tsn = ptiny.tile([1, 400], BF16, tag="tsn")
                nc.vector.tensor_scalar_mul(tsn[:], ps_ts[0:1, 0:400], rts[:])
                ps_tsb = ptp_ps.tile([128, 400], F32, tag="tsb")
                nc.tensor.matmul(
                    ps_tsb[:], lhsT=onesb[:], rhs=tsn[:], start=True, stop=True
                )
                tsb = pout.tile([128, 400], BF16, tag="tsb")
                nc.vector.tensor_copy(tsb[:], ps_tsb[:])
                nc.vector.tensor_mul(
                    get_outt(b)[:, 2, :, :],
                    s["srow"][:, :, 0:400],
                    tsb[:].rearrange("p (o c) -> p o c", o=1).broadcast_to(
                        [128, 4, 400]
                    ),
                )

            def dma_out(b):
                nc.scalar.dma_start(out=out_h[b, :, 2:3], in_=st[b]["outt"][:, 2:3])
                del st[b]

            # ---- schedule ----
            dma_in(0)
            dma_consts()
            proj_pe(0)
            proj_fix(0)
            wm_scale(0)
            for b in range(BL):
                if b + 1 < BL:
                    dma_in(b + 1)
                sim_pe(b)  # PE + ACT exp
                if b > 0:
                    ts_tail_a(b - 1)  # w@S of previous batch (PE, deps ready)
                if b + 1 < BL:
                    proj_pe(b + 1)
                    proj_fix(b + 1)
                if b > 0:
                    ts_tail_b(b - 1)  # broadcast+muls (DVE chain now hidden)
                max_tree(b)  # DVE
                if b + 1 < BL:
                    wm_scale(b + 1)  # DVE (own tile; after tree(b) to avoid HoL block)
                row_max(b)  # PE transposes + DVE reduce
                if b == BL - 1:
                    ts_tail_a(b)
                    ts_tail_b(b)  # no next iteration: run now
                at_pass(b)  # PE A@T + ACT stf + DVE muls
                if b > 0:
                    dma_out(b - 1)
            dma_out(BL - 1)
    return nc


_NC_CACHE: list = []


def _get_program() -> bass.Bass:
    if not _NC_CACHE:
        nc = build_program()
        _split_multi_waits(nc)
        _NC_CACHE.append(nc)
    return _NC_CACHE[0]


def _host_shards(S: np.ndarray, T: np.ndarray, w: np.ndarray):
    """Build per-core input maps (pure layout marshalling, no math)."""
    BF = ml_dtypes.bfloat16
    ws, wt, wm = w[:D], w[D : 2 * D], w[2 * D :]
    wcols = np.zeros((128, 8), np.float32)
    wmf = np.zeros((128, 4), np.float32)
    for kc in range(4):
        p = KP[kc]
        wcols[0:p, kc] = ws[kc * 128 : kc * 128 + p]
        wcols[0:p, 4 + kc] = wt[kc * 128 : kc * 128 + p]
        wmf[0:p, kc] = wm[kc * 128 : kc * 128 + p]
    wcols = wcols.astype(BF)
    eye = np.eye(128, dtype=BF)
    onesb = np.ones((1, 128), BF)

    def rows_pmajor(X):  # [bl, 512, 400] -> [bl, 128, 4, 404], col 400 = 1.0
        bl = X.shape[0]
        out = np.zeros((bl, 4, 128, 404), np.float32)
        out[:, :, :, 0:400] = X.reshape(bl, 4, 128, 400)
        out[:, :, :, 400] = 1.0
        return out.transpose(0, 2, 1, 3)

    def t_pmajor(X):  # [bl, 512, 400] -> big [bl, 128, 3, 512] + tail [bl,16,512]
        bl = X.shape[0]
        xt = X.transpose(0, 2, 1)  # [bl, 400, 512]
        big = xt[:, 0:384, :].reshape(bl, 3, 128, 512).transpose(0, 2, 1, 3)
        tail = xt[:, 384:400, :]
        return big, tail

    in_maps = []
    for c in range(N_CORES):
        Sb = S[c * BL : (c + 1) * BL]
        Tb = T[c * BL : (c + 1) * BL]
        rows = np.stack([rows_pmajor(Sb), rows_pmajor(Tb)], axis=2)  # [bl,128,2,4,404]
        stbig, st3 = t_pmajor(Sb)
        ttbig, tt3 = t_pmajor(Tb)
        tmaj = np.stack([stbig, ttbig], axis=2)  # [bl, 128, 2, 3, 512]
        tails = np.stack([st3, tt3], axis=1)  # [bl, 2, 16, 512]
        in_maps.append(
            {
                "rows": np.ascontiguousarray(rows).astype(BF),
                "tmaj": np.ascontiguousarray(tmaj).astype(BF),
                "tails": np.ascontiguousarray(tails).astype(BF),
                "wcols": wcols,
                "wmf": wmf,
                "eye": eye,
                "onesb": onesb,
            }
        )
    return in_maps


def kernel(source_embedding, target_embedding, w_sim, **run_kwargs):
    S = np.asarray(source_embedding, dtype=np.float32)
    T = np.asarray(target_embedding, dtype=np.float32)
    w = np.asarray(w_sim, dtype=np.float32)
    assert S.shape == (B, LS, D) and T.shape == (B, LT, D) and w.shape == (3 * D,)

    nc = _get_program()
    in_maps = _host_shards(S, T, w)
    res = run_bass_kernel_spmd(nc, in_maps, core_ids=list(range(N_CORES)), **run_kwargs)

    out = np.empty((B, LS, 4 * D), np.float32)
    out[:, :, 0:D] = S  # exact copy of the input piece (host concat)
    for c in range(N_CORES):
        piece = np.asarray(res.results[c]["outp"])  # [BL, 128, 3, 4, 400]
        sl = slice(c * BL, (c + 1) * BL)
        for q in range(3):
            out[sl, :, (q + 1) * D : (q + 2) * D] = (
                piece[:, :, q]
                .transpose(0, 2, 1, 3)
                .reshape(BL, LS, D)
                .astype(np.float32)
            )
    if run_kwargs:
        kernel.last_results = res  # expose profile info to test harness
    return out
